# revision 1
# baseline (speedup 1.0000x reference)
"""GatedCrossAttentionBlock Trainium2 kernel, SPMD over 8 NeuronCores.

Sharding: core c handles batch b=c//2, T1-half h=c%2 (1024 rows of T1).
No collectives needed. Activations kept feature-major (transposed) on device so
every matmul uses the stored weight as lhsT; per-token reductions (LN stats,
softmax sums) are done with ones-matmuls; per-token broadcasts with K=1
outer-product matmuls. All matmuls bf16 with f32 PSUM accumulation.
Host transposes per-core outputs back to token-major at the end.
"""
import sys

for _p in ("/opt/trn_rl_repo", "/root/.axon_site/_ro/trn_rl_repo"):
    if _p not in sys.path:
        sys.path.insert(0, _p)

import numpy as np
import ml_dtypes
from contextlib import ExitStack

import concourse.bass as bass
from concourse import bacc
import concourse.mybir as mybir
import concourse.tile as tile

F32 = mybir.dt.float32
BF16 = mybir.dt.bfloat16
AF = mybir.ActivationFunctionType

B, T1, TKV, N_, DIM, DL, DH, H, MULT = 4, 2048, 8, 64, 1024, 1024, 64, 8, 4
J = TKV * N_          # 512
INNER = H * DH        # 512
DFF = MULT * DIM      # 4096
TI = 1024             # T1 rows per core
NBLK = 2              # i-blocks of 512 per core
CT = DIM // 128       # 8 c-tiles
MASK_NEG = -1e4
TINY = 1e-30
EPS = 1e-5

_nc_cache = None


def build_nc():
    nc = bacc.Bacc()
    d_qoT = nc.declare_dram_parameter("qoT", [DIM, TI], F32, isOutput=False)
    d_kvoT = nc.declare_dram_parameter("kvoT", [DL, J], BF16, isOutput=False)
    d_mask = nc.declare_dram_parameter("maskTadd", [J, TI], BF16, isOutput=False)
    d_qm = nc.declare_dram_parameter("qmaskT", [1, TI], F32, isOutput=False)
    d_wg = nc.declare_dram_parameter("Wg", [DIM, INNER], BF16, isOutput=False)
    d_wqv = nc.declare_dram_parameter("wqv", [INNER, 1], F32, isOutput=False)
    d_wkv = nc.declare_dram_parameter("Wkv", [DL, 2 * INNER], BF16, isOutput=False)
    d_wout = nc.declare_dram_parameter("Woutg", [INNER, DIM], BF16, isOutput=False)
    d_w1 = nc.declare_dram_parameter("W1g", [DIM, DFF], BF16, isOutput=False)
    d_w1v = nc.declare_dram_parameter("w1v", [DFF, 1], F32, isOutput=False)
    d_w2 = nc.declare_dram_parameter("W2g", [DFF, DIM], BF16, isOutput=False)
    d_ones = nc.declare_dram_parameter("onesd", [128, 128], BF16, isOutput=False)
    d_out = nc.declare_dram_parameter("out", [DIM, TI], F32, isOutput=True)

    with tile.TileContext(nc) as tc, ExitStack() as ctx:
        pers = ctx.enter_context(tc.tile_pool(name="pers", bufs=1))
        # ---------------- persistent tiles ----------------
        qoT = [pers.tile([128, TI], F32, tag=f"qoT{t}", name=f"qoT{t}")
               for t in range(CT)]
        for t in range(CT):
            nc.sync.dma_start(out=qoT[t], in_=d_qoT[t * 128:(t + 1) * 128, :])
        ones_c = pers.tile([128, 1], BF16, tag="ones_c", name="ones_c")
        nc.sync.dma_start(out=ones_c, in_=d_ones[:, 0:1])
        ones_r = pers.tile([1, 128], BF16, tag="ones_r", name="ones_r")
        nc.sync.dma_start(out=ones_r, in_=d_ones[0:1, :])
        wq_sb = pers.tile([128, 4], F32, tag="wq_sb", name="wq_sb")
        nc.sync.dma_start(out=wq_sb, in_=d_wqv.rearrange("(t p) o -> p (t o)", p=128))
        w1_sb = pers.tile([128, 32], F32, tag="w1_sb", name="w1_sb")
        nc.sync.dma_start(out=w1_sb, in_=d_w1v.rearrange("(t p) o -> p (t o)", p=128))
        qm_sb = pers.tile([1, TI], F32, tag="qm_sb", name="qm_sb")
        nc.sync.dma_start(out=qm_sb, in_=d_qm[:, :])
        eps_t = pers.tile([1, 1], F32, tag="eps_t", name="eps_t")
        nc.vector.memset(eps_t[:], EPS)
        xT = [pers.tile([128, TI], F32, tag=f"xT{t}", name=f"xT{t}")
              for t in range(CT)]
        xc = [pers.tile([128, TI], BF16, tag=f"xc{t}", name=f"xc{t}")
              for t in range(CT)]

        scr3 = ctx.enter_context(tc.tile_pool(name="scr3", bufs=3))

        def ln_stats(pa, ps_stat, src_tiles, tag):
            """Per-token (free-dim) mean/rstd of a feature-major (DIM, TI)
            activation held as 8 (128, TI) tiles. Returns mean broadcast
            PSUM tiles (one per 512-block) and rstd broadcast in SBUF."""
            mu_ps = [ps_stat.tile([1, 512], F32, tag=f"mu{b}", name=f"mu{tag}{b}")
                     for b in range(NBLK)]
            ss_ps = [ps_stat.tile([1, 512], F32, tag=f"ss{b}", name=f"ss{tag}{b}")
                     for b in range(NBLK)]
            for t in range(CT):
                cbf = scr3.tile([128, TI], BF16, tag="statbf", name="statbf", bufs=2)
                nc.vector.tensor_copy(cbf[:], src_tiles[t][:])
                sq = scr3.tile([128, TI], BF16, tag="statsq", name="statsq", bufs=2)
                nc.scalar.square(sq[:], src_tiles[t][:])
                for b in range(NBLK):
                    sl = slice(b * 512, b * 512 + 512)
                    nc.tensor.matmul(mu_ps[b][:], ones_c[:], cbf[:, sl],
                                     start=(t == 0), stop=(t == CT - 1))
                    nc.tensor.matmul(ss_ps[b][:], ones_c[:], sq[:, sl],
                                     start=(t == 0), stop=(t == CT - 1))
            mu = pa.tile([1, TI], F32, tag="st_mu", name=f"mu{tag}")
            ex2 = pa.tile([1, TI], F32, tag="st_ex2", name=f"ex2{tag}")
            for b in range(NBLK):
                sl = slice(b * 512, b * 512 + 512)
                nc.vector.tensor_scalar_mul(mu[:, sl], mu_ps[b][:], 1.0 / DIM)
                nc.vector.tensor_scalar_mul(ex2[:, sl], ss_ps[b][:], 1.0 / DIM)
            mu_bf = pa.tile([1, TI], BF16, tag="st_mubf", name=f"mubf{tag}")
            nc.vector.tensor_copy(mu_bf[:], mu[:])
            musq = pa.tile([1, TI], F32, tag="st_musq", name=f"musq{tag}")
            nc.vector.tensor_mul(musq[:], mu[:], mu[:])
            var = pa.tile([1, TI], F32, tag="st_mu", name=f"var{tag}")
            nc.vector.tensor_sub(var[:], ex2[:], musq[:])
            std = pa.tile([1, TI], F32, tag="st_musq", name=f"std{tag}")
            nc.scalar.activation(std[:], var[:], AF.Sqrt, bias=eps_t[:])
            r = pa.tile([1, TI], F32, tag="st_ex2", name=f"r{tag}")
            nc.vector.reciprocal(r[:], std[:])
            r_bf = pa.tile([1, TI], BF16, tag="st_rbf", name=f"rbf{tag}")
            nc.vector.tensor_copy(r_bf[:], r[:])
            mu_b = [ps_stat.tile([128, 512], F32, tag=f"mu{b}", name=f"mub{tag}{b}")
                    for b in range(NBLK)]
            rb_sb = pa.tile([128, TI], F32, tag="st_rbsb", name=f"rbsb{tag}")
            for b in range(NBLK):
                sl = slice(b * 512, b * 512 + 512)
                nc.tensor.matmul(mu_b[b][:], ones_r[:], mu_bf[:, sl],
                                 start=True, stop=True)
                rb_ps = ps_stat.tile([128, 512], F32, tag=f"ss{b}",
                                     name=f"rbps{tag}{b}")
                nc.tensor.matmul(rb_ps[:], ones_r[:], r_bf[:, sl],
                                 start=True, stop=True)
                nc.vector.tensor_copy(rb_sb[:, sl], rb_ps[:])
            return mu_b, rb_sb

        def normalize(pa, ps_stat, src_tiles, tag):
            mu_b, rb_sb = ln_stats(pa, ps_stat, src_tiles, tag)
            for t in range(CT):
                for b in range(NBLK):
                    sl = slice(b * 512, b * 512 + 512)
                    d = scr3.tile([128, 512], F32, tag="xcscr", name="xcscr", bufs=2)
                    nc.vector.tensor_sub(d[:], src_tiles[t][:, sl], mu_b[b][:])
                    nc.vector.tensor_mul(xc[t][:, sl], d[:], rb_sb[:, sl])

        with tc.tile_pool(name="attn", bufs=1) as pa:
            wg_sb = [pa.tile([128, INNER], BF16, tag=f"wg{t}", name=f"wg{t}")
                     for t in range(CT)]
            wkv_sb = [pa.tile([128, 2 * INNER], BF16, tag=f"wkv{t}", name=f"wkv{t}")
                      for t in range(CT)]
            kvoT = [pa.tile([128, J], BF16, tag=f"kvo{t}", name=f"kvo{t}")
                    for t in range(CT)]
            mask_sb = [pa.tile([128, TI], BF16, tag=f"mask{t}", name=f"mask{t}")
                       for t in range(4)]
            for t in range(CT):
                nc.sync.dma_start(out=wg_sb[t], in_=d_wg[t * 128:(t + 1) * 128, :])
                nc.sync.dma_start(out=wkv_sb[t], in_=d_wkv[t * 128:(t + 1) * 128, :])
                nc.sync.dma_start(out=kvoT[t], in_=d_kvoT[t * 128:(t + 1) * 128, :])
            for t in range(4):
                nc.sync.dma_start(out=mask_sb[t], in_=d_mask[t * 128:(t + 1) * 128, :])

            qT = [pa.tile([128, TI], BF16, tag=f"qT{d}", name=f"qT{d}")
                  for d in range(4)]
            kT = [pa.tile([128, J], BF16, tag=f"kT{d}", name=f"kT{d}")
                  for d in range(4)]
            v_aug = [pa.tile([128, H, DH + 1], BF16, tag=f"vaug{j}", name=f"vaug{j}")
                     for j in range(4)]

            # ---- scope A: LN1 + q/k/v projections ----
            with tc.tile_pool(name="psA", bufs=1, space="PSUM") as psA, \
                 tc.tile_pool(name="psAcc", bufs=2, space="PSUM") as psAcc:
                normalize(pa, psA, qoT, "1")
                for d in range(4):
                    for b in range(NBLK):
                        sl = slice(b * 512, b * 512 + 512)
                        q_ps = psAcc.tile([128, 512], F32, tag="acc", name="q_ps")
                        for t in range(CT):
                            nc.tensor.matmul(q_ps[:],
                                             wg_sb[t][:, d * 128:(d + 1) * 128],
                                             xc[t][:, sl],
                                             start=(t == 0), stop=(t == CT - 1))
                        nc.vector.tensor_scalar_add(qT[d][:, sl], q_ps[:],
                                                    wq_sb[:, d:d + 1])
                for d in range(4):
                    k_ps = psAcc.tile([128, 512], F32, tag="acc", name="k_ps")
                    for t in range(CT):
                        nc.tensor.matmul(k_ps[:],
                                         wkv_sb[t][:, d * 128:(d + 1) * 128],
                                         kvoT[t][:], start=(t == 0),
                                         stop=(t == CT - 1))
                    nc.vector.tensor_copy(kT[d][:], k_ps[:])
                for j in range(4):
                    v_ps = psAcc.tile([128, 512], F32, tag="acc", name="v_ps")
                    for t in range(CT):
                        nc.tensor.matmul(v_ps[:],
                                         kvoT[t][:, j * 128:(j + 1) * 128],
                                         wkv_sb[t][:, INNER:2 * INNER],
                                         start=(t == 0), stop=(t == CT - 1))
                    nc.vector.tensor_copy(
                        v_aug[j][:, :, 0:DH],
                        v_ps[:].rearrange("p (h d) -> p h d", h=H))
                    nc.vector.memset(v_aug[j][:, :, DH:DH + 1], 1.0)

            # ---- scope B: attention ----
            attn_cat = [pa.tile([128, TI], BF16, tag=f"wkv{d}", name=f"acat{d}")
                        for d in range(4)]
            with tc.tile_pool(name="psSim", bufs=3, space="PSUM") as psSim, \
                 tc.tile_pool(name="psAv", bufs=2, space="PSUM") as psAv:
                for h in range(H):
                    dt_h, row = h // 2, 64 * (h % 2)
                    for b in range(NBLK):
                        sl = slice(b * 512, b * 512 + 512)
                        pT = []
                        for j in range(4):
                            s_ps = psSim.tile([128, 512], F32, tag="sim",
                                              name="s_ps")
                            nc.tensor.matmul(
                                s_ps[:],
                                kT[dt_h][row:row + 64, j * 128:(j + 1) * 128],
                                qT[dt_h][row:row + 64, sl],
                                start=True, stop=True)
                            tf = scr3.tile([128, 512], F32, tag="expin",
                                           name="expin", bufs=2)
                            nc.vector.tensor_add(tf[:], s_ps[:],
                                                 mask_sb[j][:, sl])
                            p = scr3.tile([128, 512], BF16, tag="pT", name="pT")
                            nc.scalar.activation(p[:], tf[:], AF.Exp)
                            pT.append(p)
                        av_ps = psAv.tile([DH + 1, 512], F32, tag="av",
                                          name="av_ps")
                        for j in range(4):
                            nc.tensor.matmul(av_ps[:], v_aug[j][:, h, :],
                                             pT[j][:], start=(j == 0),
                                             stop=(j == 3))
                        s_t = scr3.tile([1, 512], F32, tag="s_t", name="s_t", bufs=2)
                        nc.vector.tensor_scalar_add(s_t[:], av_ps[DH:DH + 1, :],
                                                    TINY)
                        rec = scr3.tile([1, 512], F32, tag="rec", name="rec", bufs=2)
                        nc.vector.reciprocal(rec[:], s_t[:])
                        r_bf = scr3.tile([1, 512], BF16, tag="rbf_h", name="rbf_h", bufs=2)
                        nc.vector.tensor_mul(r_bf[:], rec[:], qm_sb[:, sl])
                        rb_ps = psAv.tile([64, 512], F32, tag="rb", name="rb_ps")
                        nc.tensor.matmul(rb_ps[:], ones_r[:, 0:64], r_bf[:],
                                         start=True, stop=True)
                        rb2 = scr3.tile([64, 512], F32, tag="rb2", name="rb2", bufs=2)
                        nc.vector.tensor_copy(rb2[:], rb_ps[:])
                        nc.vector.tensor_mul(attn_cat[dt_h][row:row + 64, sl],
                                             av_ps[0:DH, :], rb2[:])

            # ---- scope C: Wout + gated residual ----
            wor = d_wout.rearrange("(t p) n -> p t n", p=128)
            with tc.tile_pool(name="psC", bufs=3, space="PSUM") as psC:
                for e in range(CT):
                    wot = scr3.tile([128, 4, 128], BF16, tag="wos", name="wot",
                                    bufs=2)
                    nc.sync.dma_start(out=wot,
                                      in_=wor[:, :, e * 128:(e + 1) * 128])
                    for b in range(NBLK):
                        sl = slice(b * 512, b * 512 + 512)
                        wo_ps = psC.tile([128, 512], F32, tag="acc", name="wo_ps")
                        for d in range(4):
                            nc.tensor.matmul(wo_ps[:], wot[:, d, :],
                                             attn_cat[d][:, sl],
                                             start=(d == 0), stop=(d == 3))
                        nc.vector.tensor_add(xT[e][:, sl], wo_ps[:],
                                             qoT[e][:, sl])

            # ---- scope D: LN2 -> xc2 (reuses xc tiles) ----
            with tc.tile_pool(name="psD", bufs=1, space="PSUM") as psD:
                normalize(pa, psD, xT, "2")

        # ---------------- FFN ----------------
        with tc.tile_pool(name="ffn", bufs=1) as pf, \
             tc.tile_pool(name="wstream", bufs=2) as ws, \
             tc.tile_pool(name="ostage", bufs=2) as ost, \
             tc.tile_pool(name="psH", bufs=2, space="PSUM") as psH:
            gT = [pf.tile([128, TI], BF16, tag=f"gT{f}", name=f"gT{f}")
                  for f in range(32)]
            w1r = d_w1.rearrange("(t p) n -> p t n", p=128)
            w2r = d_w2.rearrange("(t p) n -> p t n", p=128)
            for f in range(32):
                w1t = ws.tile([128, CT, 128], BF16, tag="w1s", name="w1t")
                nc.sync.dma_start(out=w1t, in_=w1r[:, :, f * 128:(f + 1) * 128])
                for b in range(NBLK):
                    sl = slice(b * 512, b * 512 + 512)
                    h1_ps = psH.tile([128, 512], F32, tag="h1", name="h1_ps")
                    for t in range(CT):
                        nc.tensor.matmul(h1_ps[:], w1t[:, t, :], xc[t][:, sl],
                                         start=(t == 0), stop=(t == CT - 1))
                    nc.scalar.activation(gT[f][:, sl], h1_ps[:], AF.Gelu,
                                         bias=w1_sb[:, f:f + 1])
            for e in range(CT):
                w2t = ws.tile([128, 32, 128], BF16, tag="w2s", name="w2t")
                nc.sync.dma_start(out=w2t, in_=w2r[:, :, e * 128:(e + 1) * 128])
                for b in range(NBLK):
                    sl = slice(b * 512, b * 512 + 512)
                    h2_ps = psH.tile([128, 512], F32, tag="h2", name="h2_ps")
                    for t in range(32):
                        nc.tensor.matmul(h2_ps[:], w2t[:, t, :], gT[t][:, sl],
                                         start=(t == 0), stop=(t == 31))
                    stg = ost.tile([128, 512], F32, tag="stg", name="stg")
                    nc.vector.tensor_add(stg[:], h2_ps[:], xT[e][:, sl])
                    nc.sync.dma_start(out=d_out[e * 128:(e + 1) * 128, sl],
                                      in_=stg[:])
    nc.compile()
    return nc


def _prep_in_maps(qo, kvo, attn_mask, q_mask, kv_mask, ln_g, ln_b, Wq, Wkv, Wout,
                  attn_gate, ff_ln_g, ff_ln_b, W1, W2, ff_gate):
    bf = ml_dtypes.bfloat16
    scale = DH ** (-0.5)
    tanh_a = float(np.tanh(np.float32(attn_gate[0])))
    tanh_f = float(np.tanh(np.float32(ff_gate[0])))
    Wg = ln_g[:, None].astype(np.float64) * Wq.astype(np.float64) * scale
    wqv = ln_b.astype(np.float64) @ Wq.astype(np.float64) * scale
    W1g = ff_ln_g[:, None].astype(np.float64) * W1.astype(np.float64)
    w1v = ff_ln_b.astype(np.float64) @ W1.astype(np.float64)
    shared = {
        "Wg": np.ascontiguousarray(Wg, dtype=bf),
        "wqv": np.ascontiguousarray(wqv[:, None], dtype=np.float32),
        "Wkv": np.ascontiguousarray(Wkv, dtype=bf),
        "Woutg": np.ascontiguousarray(Wout.astype(np.float64) * tanh_a, dtype=bf),
        "W1g": np.ascontiguousarray(W1g, dtype=bf),
        "w1v": np.ascontiguousarray(w1v[:, None], dtype=np.float32),
        "W2g": np.ascontiguousarray(W2.astype(np.float64) * tanh_f, dtype=bf),
        "onesd": np.ones((128, 128), dtype=bf),
    }
    in_maps = []
    for c in range(8):
        b, hf = c // 2, c % 2
        rows = slice(hf * TI, (hf + 1) * TI)
        m = attn_mask[b, rows, :] & kv_mask[b].reshape(J)[None, :]
        maskTadd = np.where(m.T, 0.0, MASK_NEG).astype(bf)
        im = dict(shared)
        im["qoT"] = np.ascontiguousarray(qo[b, rows, :].T, dtype=np.float32)
        im["kvoT"] = np.ascontiguousarray(kvo[b].reshape(J, DL).T, dtype=bf)
        im["maskTadd"] = np.ascontiguousarray(maskTadd)
        im["qmaskT"] = np.ascontiguousarray(q_mask[b, rows][None, :],
                                            dtype=np.float32)
        in_maps.append(im)
    return in_maps


def kernel(**inputs):
    global _nc_cache
    inputs = {k: np.asarray(v) for k, v in inputs.items()}
    in_maps = _prep_in_maps(**inputs)
    if _nc_cache is None:
        _nc_cache = build_nc()
    from concourse.bass_utils import run_bass_kernel_spmd
    res = run_bass_kernel_spmd(_nc_cache, in_maps, list(range(8)))
    out = np.empty((B, T1, DIM), dtype=np.float32)
    for c in range(8):
        b, hf = c // 2, c % 2
        out[b, hf * TI:(hf + 1) * TI, :] = res.results[c]["out"].T
    return out


if __name__ == "__main__":
    nc = build_nc()
    print("built ok")



# revision 13
# speedup vs baseline: 1.4315x; 1.4315x over previous
"""GatedCrossAttentionBlock Trainium2 kernel, SPMD over 8 NeuronCores.

Sharding: core c handles batch b=c//2, T1-half h=c%2 (1024 rows of T1).
No collectives. Activations feature-major (transposed); all big matmuls
fp8e4 DoubleRow (2x tensor throughput), accumulating f32 in PSUM.

Scale folding: the whole post-attention residual stream is carried
S2-scaled (S2 a power of two) so Wout/W2 quantization scales cost no
extra ops; host divides the output by S2. LayerNorm mean-subtraction is
folded into the projection matmuls as a rank-1 update (colsum(W) x
mu*rstd), so normalize is a single vector multiply per tile.
"""
import sys

for _p in ("/opt/trn_rl_repo", "/root/.axon_site/_ro/trn_rl_repo"):
    if _p not in sys.path:
        sys.path.insert(0, _p)

import numpy as np
import ml_dtypes
from contextlib import ExitStack

import concourse.bass as bass
from concourse import bacc
import concourse.mybir as mybir
import concourse.tile as tile

F32 = mybir.dt.float32
BF16 = mybir.dt.bfloat16
FP8 = mybir.dt.float8e4
AF = mybir.ActivationFunctionType
ALU = mybir.AluOpType
DR = mybir.MatmulPerfMode.DoubleRow

B, T1, TKV, N_, DIM, DL, DH, H, MULT = 4, 2048, 8, 64, 1024, 1024, 64, 8, 4
J = TKV * N_          # 512
INNER = H * DH        # 512
DFF = MULT * DIM      # 4096
TI = 1024             # T1 rows per core
NBLK = 2              # i-blocks of 512 per core
CT = DIM // 128       # 8 c-tiles
TINY = 1e-30
EPS = 1e-5

_nc_cache = None
_nc_key = None


def build_nc(SQ, SKV, S1, S2):
    nc = bacc.Bacc()
    d_qoT = nc.declare_dram_parameter("qoT", [DIM, TI], BF16, isOutput=False)
    d_kvq = nc.declare_dram_parameter("kvq", [128, 8 * J], FP8, isOutput=False)
    d_mask = nc.declare_dram_parameter("mask01", [128, 4 * TI], FP8,
                                       isOutput=False)
    d_qm = nc.declare_dram_parameter("qmaskT", [1, TI], F32, isOutput=False)
    d_wgq = nc.declare_dram_parameter("wgq", [128, 8 * INNER], FP8,
                                      isOutput=False)
    d_cwg = nc.declare_dram_parameter("cwg", [1, INNER], BF16, isOutput=False)
    d_wqv = nc.declare_dram_parameter("wqv", [128, 4], F32, isOutput=False)
    d_wkvq = nc.declare_dram_parameter("wkvq", [128, 8 * 2 * INNER], FP8,
                                       isOutput=False)
    d_woq = nc.declare_dram_parameter("woq", [128, 4 * DIM], FP8,
                                      isOutput=False)
    d_w1q = nc.declare_dram_parameter("w1q", [128, 8 * DFF], FP8,
                                      isOutput=False)
    d_cw1 = nc.declare_dram_parameter("cw1", [1, DFF], BF16, isOutput=False)
    d_w1v = nc.declare_dram_parameter("w1v", [128, 32], F32, isOutput=False)
    d_w2q = nc.declare_dram_parameter("w2q", [128, 8 * 32 * 128], FP8,
                                      isOutput=False)
    d_out = nc.declare_dram_parameter("out", [DIM, TI], F32, isOutput=True)

    with tile.TileContext(nc) as tc, ExitStack() as ctx:
        pers = ctx.enter_context(tc.tile_pool(name="pers", bufs=1))
        # ---------------- persistent tiles ----------------
        xT = [pers.tile([128, TI], F32, tag=f"xT{t}", name=f"xT{t}")
              for t in range(CT)]
        # LN output, fp8, DoubleRow layout: tile tp holds chunks 2tp, 2tp+1.
        xc8 = [pers.tile([128, 2, TI], FP8, tag=f"xc{t}", name=f"xc{t}")
               for t in range(4)]
        w1q_sb = pers.tile([128, 8, DFF], FP8, tag="w1q", name="w1q_sb")
        cwg_sb = pers.tile([1, INNER], BF16, tag="cwg", name="cwg_sb")
        cw1_sb = pers.tile([1, DFF], BF16, tag="cw1", name="cw1_sb")
        wqv_sb = pers.tile([128, 4], F32, tag="wqv", name="wqv_sb")
        w1v_sb = pers.tile([128, 32], F32, tag="w1v", name="w1v_sb")
        qm_sb = pers.tile([1, TI], F32, tag="qm", name="qm_sb")
        ones_c = pers.tile([128, 1], BF16, tag="ones_c", name="ones_c")
        ones_r = pers.tile([1, 128], BF16, tag="ones_r", name="ones_r")
        eps_t = pers.tile([1, 1], F32, tag="eps_t", name="eps_t")
        negmurb = [pers.tile([1, TI], BF16, tag=f"nmr{i}", name=f"nmr{i}")
                   for i in range(2)]
        nc.vector.memset(ones_c[:], 1.0)
        nc.vector.memset(ones_r[:], 1.0)
        nc.vector.memset(eps_t[:], EPS * S2 * S2)

        scr = ctx.enter_context(tc.tile_pool(name="scr", bufs=3))

        def ln_rank1(pa, ps_stat, src_tiles, src_bf, rb_sb, nmr, tag):
            """Stats of a feature-major (DIM, TI) S2-scaled activation.
            Writes rb_sb [128, TI] f32 (rstd broadcast) and nmr [1, TI]
            bf16 (-mu*rstd) for the rank-1 mean correction."""
            st = [ps_stat.tile([33, 512], F32, tag=f"stat{b}",
                               name=f"st{tag}{b}") for b in range(NBLK)]
            for t in range(CT):
                if src_bf:
                    cbf = src_tiles[t]
                else:
                    cbf = scr.tile([128, TI], BF16, tag="statbf",
                                   name="statbf", bufs=2)
                    nc.vector.tensor_copy(cbf[:], src_tiles[t][:])
                sq = scr.tile([128, TI], BF16, tag="statsq", name="statsq",
                              bufs=2)
                nc.scalar.square(sq[:], cbf[:])
                for b in range(NBLK):
                    sl = slice(b * 512, b * 512 + 512)
                    nc.tensor.matmul(st[b][0:1, :], ones_c[:], cbf[:, sl],
                                     start=(t == 0), stop=(t == CT - 1))
                    nc.tensor.matmul(st[b][32:33, :], ones_c[:],
                                     sq[:, sl], start=(t == 0),
                                     stop=(t == CT - 1))
            mu = pa.tile([1, TI], F32, tag="st_mu", name=f"mu{tag}")
            ex2 = pa.tile([1, TI], F32, tag="st_ex2", name=f"ex2{tag}")
            for b in range(NBLK):
                sl = slice(b * 512, b * 512 + 512)
                nc.vector.tensor_scalar_mul(mu[:, sl], st[b][0:1, :],
                                            1.0 / DIM)
                nc.vector.tensor_scalar_mul(ex2[:, sl], st[b][32:33, :],
                                            1.0 / DIM)
            musq = pa.tile([1, TI], F32, tag="st_musq", name=f"musq{tag}")
            nc.vector.tensor_mul(musq[:], mu[:], mu[:])
            var = pa.tile([1, TI], F32, tag="st_var", name=f"var{tag}")
            nc.vector.tensor_sub(var[:], ex2[:], musq[:])
            std = pa.tile([1, TI], F32, tag="st_musq", name=f"std{tag}")
            nc.scalar.activation(std[:], var[:], AF.Sqrt, bias=eps_t[:])
            r = pa.tile([1, TI], F32, tag="st_ex2", name=f"r{tag}")
            nc.vector.reciprocal(r[:], std[:])
            r_bf = pa.tile([1, TI], BF16, tag="st_rbf", name=f"rbf{tag}")
            nc.vector.tensor_copy(r_bf[:], r[:])
            # nmr = -mu * rstd  (bf16)
            nmrf = pa.tile([1, TI], F32, tag="st_var", name=f"nmrf{tag}")
            nc.vector.tensor_mul(nmrf[:], mu[:], r[:])
            nc.vector.tensor_scalar_mul(nmr[:], nmrf[:], -1.0)
            for b in range(NBLK):
                sl = slice(b * 512, b * 512 + 512)
                rb_ps = ps_stat.tile([128, 512], F32, tag="rbb",
                                     name=f"rbps{tag}{b}", bufs=2)
                nc.tensor.matmul(rb_ps[:], ones_r[:], r_bf[:, sl],
                                 start=True, stop=True)
                nc.vector.tensor_copy(rb_sb[:, sl], rb_ps[:])

        def norm_mul(src_tiles, rb_sb):
            for t in range(CT):
                nc.vector.tensor_mul(xc8[t // 2][:, t % 2, :],
                                     src_tiles[t][:], rb_sb[:])

        with tc.tile_pool(name="attn", bufs=1) as pa:
            qoT = [pa.tile([128, TI], BF16, tag=f"qoT{t}", name=f"qoT{t}")
                   for t in range(CT)]
            kv_sb = pa.tile([128, 8, J], FP8, tag="kv", name="kv_sb")
            mask_sb = pa.tile([128, 4, TI], FP8, tag="mask", name="mask_sb")
            wgq_sb = pa.tile([128, 8, INNER], FP8, tag="wgq", name="wgq_sb")
            wkvq_sb = pa.tile([128, 8, 2 * INNER], FP8, tag="wkvq",
                              name="wkvq_sb")
            woq_sb = pa.tile([128, 4, DIM], FP8, tag="woq", name="woq_sb")
            rb1_sb = pa.tile([128, TI], F32, tag="rb1", name="rb1_sb")
            rb2_sb = pa.tile([128, TI], F32, tag="rb1", name="rb2_sb")
            qT = [pa.tile([128, TI], BF16, tag=f"qT{d}", name=f"qT{d}")
                  for d in range(4)]
            kT = [pa.tile([128, J], BF16, tag=f"kT{d}", name=f"kT{d}")
                  for d in range(4)]
            # per-head stride padded to 72 so DoubleRow ldweights APs stay
            # even-sized and even-aligned (65 is rejected by codegen)
            VP = 72
            v_aug = [pa.tile([128, 2, H, VP], FP8, tag=f"vaug{j}",
                             name=f"vaug{j}") for j in range(2)]
            attn_cat = [pa.tile([128, 2, TI], FP8, tag=f"acat{d}",
                                name=f"acat{d}") for d in range(2)]

            for t in range(CT):
                nc.sync.dma_start(out=qoT[t],
                                  in_=d_qoT[t * 128:(t + 1) * 128, :])
            nc.sync.dma_start(out=kv_sb,
                              in_=d_kvq.rearrange("p (a j) -> p a j", a=8))
            nc.sync.dma_start(out=mask_sb,
                              in_=d_mask.rearrange("p (a t) -> p a t", a=4))
            nc.sync.dma_start(out=qm_sb, in_=d_qm[:, :])
            nc.sync.dma_start(out=wgq_sb,
                              in_=d_wgq.rearrange("p (a n) -> p a n", a=8))
            nc.sync.dma_start(out=wkvq_sb,
                              in_=d_wkvq.rearrange("p (a n) -> p a n", a=8))
            nc.sync.dma_start(out=cwg_sb, in_=d_cwg[:, :])
            nc.sync.dma_start(out=wqv_sb, in_=d_wqv[:, :])
            nc.sync.dma_start(out=cw1_sb, in_=d_cw1[:, :])
            nc.sync.dma_start(out=w1v_sb, in_=d_w1v[:, :])
            nc.sync.dma_start(out=woq_sb,
                              in_=d_woq.rearrange("p (a n) -> p a n", a=4))
            nc.sync.dma_start(out=w1q_sb,
                              in_=d_w1q.rearrange("p (a n) -> p a n", a=8))

            for jp in range(2):
                nc.vector.memset(v_aug[jp][:, :, :, DH:DH + 1], 1.0)
                nc.vector.memset(v_aug[jp][:, :, :, DH + 1:VP], 0.0)

            # ---- LN1 stats + k/v projections ----
            with tc.tile_pool(name="psStat", bufs=1, space="PSUM") as psStat, \
                 tc.tile_pool(name="psKV", bufs=2, space="PSUM") as psKV:
                ln_rank1(pa, psStat, qoT, True, rb1_sb, negmurb[0], "1")
                for d in range(4):
                    k_ps = psKV.tile([128, 512], F32, tag="kv", name="k_ps")
                    for tp in range(4):
                        nc.tensor.matmul(
                            k_ps[:],
                            wkvq_sb[:, 2 * tp:2 * tp + 2,
                                    d * 128:(d + 1) * 128],
                            kv_sb[:, 2 * tp:2 * tp + 2, :],
                            start=(tp == 0), stop=(tp == 3), perf_mode=DR)
                    nc.scalar.activation(kT[d][:], k_ps[:], AF.Copy)
                for c in range(4):
                    v_ps = psKV.tile([128, 512], F32, tag="kv", name="v_ps")
                    for tp in range(4):
                        nc.tensor.matmul(
                            v_ps[:],
                            kv_sb[:, 2 * tp:2 * tp + 2,
                                  c * 128:(c + 1) * 128],
                            wkvq_sb[:, 2 * tp:2 * tp + 2, INNER:2 * INNER],
                            start=(tp == 0), stop=(tp == 3), perf_mode=DR)
                    nc.vector.tensor_scalar_mul(
                        v_aug[c // 2][:, c % 2, :, 0:DH],
                        v_ps[:].rearrange("p (h d) -> p h d", h=H),
                        1.0 / SKV)
            norm_mul(qoT, rb1_sb)

            # ---- q projection ----
            with tc.tile_pool(name="psQ", bufs=2, space="PSUM") as psQ:
                for d in range(4):
                    q_ps = psQ.tile([128, 2, 512], F32, tag="q", name="q_ps")
                    for b in range(NBLK):
                        sl = slice(b * 512, b * 512 + 512)
                        for tp in range(4):
                            nc.tensor.matmul(
                                q_ps[:, b, :],
                                wgq_sb[:, 2 * tp:2 * tp + 2,
                                       d * 128:(d + 1) * 128],
                                xc8[tp][:, :, sl],
                                start=(tp == 0), stop=False, perf_mode=DR)
                        nc.tensor.matmul(q_ps[:, b, :],
                                         cwg_sb[:, d * 128:(d + 1) * 128],
                                         negmurb[0][:, sl],
                                         start=False, stop=True)
                    for b in range(NBLK):
                        sl = slice(b * 512, b * 512 + 512)
                        nc.vector.tensor_scalar(qT[d][:, sl], q_ps[:, b, :],
                                                1.0 / SQ, wqv_sb[:, d:d + 1],
                                                op0=ALU.mult, op1=ALU.add)

            # ---- attention ----
            with tc.tile_pool(name="psS", bufs=2, space="PSUM") as psS, \
                 tc.tile_pool(name="psAv", bufs=2, space="PSUM") as psAv:
                for h in range(H):
                    dt_h, row = h // 2, 64 * (h % 2)
                    for b in range(NBLK):
                        sl = slice(b * 512, b * 512 + 512)
                        av_ps = psAv.tile([VP, 512], F32, tag="av",
                                          name="av_ps")
                        for jp in range(2):
                            s_ps = psS.tile([128, 2, 512], F32, tag="sim",
                                            name="s_ps")
                            for i in range(2):
                                jc = 2 * jp + i
                                nc.tensor.matmul(
                                    s_ps[:, i, :],
                                    kT[dt_h][row:row + 64,
                                             jc * 128:(jc + 1) * 128],
                                    qT[dt_h][row:row + 64, sl],
                                    start=True, stop=True)
                            pe = scr.tile([128, 2, 512], BF16, tag="pe",
                                          name="pe", bufs=2)
                            nc.scalar.activation(pe[:], s_ps[:], AF.Exp,
                                                 scale=1.0 / SKV)
                            pq = scr.tile([128, 2, 512], FP8, tag="pq",
                                          name="pq", bufs=3)
                            nc.gpsimd.tensor_mul(
                                pq[:], pe[:], mask_sb[:, 2 * jp:2 * jp + 2, sl])
                            nc.tensor.matmul(av_ps[:],
                                             v_aug[jp][:, :, h, :], pq[:],
                                             start=(jp == 0), stop=(jp == 1),
                                             perf_mode=DR)
                        s_t = scr.tile([1, 512], F32, tag="s_t", name="s_t",
                                       bufs=2)
                        nc.vector.tensor_scalar_add(s_t[:],
                                                    av_ps[DH:DH + 1, :], TINY)
                        rec = scr.tile([1, 512], F32, tag="rec", name="rec",
                                       bufs=2)
                        nc.vector.reciprocal(rec[:], s_t[:])
                        r_bf = scr.tile([1, 512], BF16, tag="rbf_h",
                                        name="rbf_h", bufs=2)
                        nc.vector.tensor_mul(r_bf[:], rec[:], qm_sb[:, sl])
                        rb_ps = psAv.tile([64, 512], F32, tag="rb",
                                          name="rb_ps")
                        nc.tensor.matmul(rb_ps[:], ones_r[:, 0:64], r_bf[:],
                                         start=True, stop=True)
                        rb2 = scr.tile([64, 512], F32, tag="rb2", name="rb2",
                                       bufs=2)
                        nc.vector.tensor_copy(rb2[:], rb_ps[:])
                        nc.vector.tensor_mul(
                            attn_cat[h // 4][row:row + 64, (h // 2) % 2, sl],
                            av_ps[0:DH, :], rb2[:])

            # ---- Wout + residual, then LN2 ----
            with tc.tile_pool(name="psC", bufs=2, space="PSUM") as psC, \
                 tc.tile_pool(name="psStat2", bufs=1, space="PSUM") as psS2:
                for b in range(NBLK):
                    sl = slice(b * 512, b * 512 + 512)
                    for e in range(CT):
                        wo_ps = psC.tile([128, 512], F32, tag="wo",
                                         name="wo_ps")
                        for dp in range(2):
                            nc.tensor.matmul(
                                wo_ps[:],
                                woq_sb[:, 2 * dp:2 * dp + 2,
                                       e * 128:(e + 1) * 128],
                                attn_cat[dp][:, :, sl],
                                start=(dp == 0), stop=(dp == 1), perf_mode=DR)
                        nc.vector.tensor_add(xT[e][:, sl], wo_ps[:],
                                             qoT[e][:, sl])
                ln_rank1(pa, psS2, xT, False, rb2_sb, negmurb[1], "2")
            norm_mul(xT, rb2_sb)

        # ---------------- FFN ----------------
        with tc.tile_pool(name="ffn", bufs=1) as pf, \
             tc.tile_pool(name="wstream", bufs=2) as ws, \
             tc.tile_pool(name="ostage", bufs=2) as ost, \
             tc.tile_pool(name="psH", bufs=3, space="PSUM") as psH:
            gT8 = [pf.tile([128, 2, TI], FP8, tag=f"gT{f}", name=f"gT{f}")
                   for f in range(16)]
            for f in range(32):
                h_ps = psH.tile([128, 2, 512], F32, tag="h", name="h_ps")
                for b in range(NBLK):
                    sl = slice(b * 512, b * 512 + 512)
                    for tp in range(4):
                        nc.tensor.matmul(
                            h_ps[:, b, :],
                            w1q_sb[:, 2 * tp:2 * tp + 2,
                                   f * 128:(f + 1) * 128],
                            xc8[tp][:, :, sl],
                            start=(tp == 0), stop=False, perf_mode=DR)
                    nc.tensor.matmul(h_ps[:, b, :],
                                     cw1_sb[:, f * 128:(f + 1) * 128],
                                     negmurb[1][:, sl],
                                     start=False, stop=True)
                nc.scalar.activation(
                    gT8[f // 2][:, f % 2, :],
                    h_ps[:].rearrange("p a t -> p (a t)"),
                    AF.Gelu, bias=w1v_sb[:, f:f + 1], scale=1.0 / S1)
            w2r = d_w2q.rearrange("p (e a n) -> p e a n", e=8, a=32)
            for e in range(CT):
                w2t = ws.tile([128, 32, 128], FP8, tag="w2s", name="w2t")
                nc.sync.dma_start(out=w2t, in_=w2r[:, e])
                h2_ps = psH.tile([128, 2, 512], F32, tag="h", name="h2_ps")
                for b in range(NBLK):
                    sl = slice(b * 512, b * 512 + 512)
                    for fp in range(16):
                        nc.tensor.matmul(
                            h2_ps[:, b, :],
                            w2t[:, 2 * fp:2 * fp + 2, :],
                            gT8[fp][:, :, sl],
                            start=(fp == 0), stop=(fp == 15), perf_mode=DR)
                stg = ost.tile([128, TI], F32, tag="stg", name="stg")
                nc.vector.tensor_add(stg[:],
                                     h2_ps[:].rearrange("p a t -> p (a t)"),
                                     xT[e][:])
                nc.sync.dma_start(out=d_out[e * 128:(e + 1) * 128, :],
                                  in_=stg[:])
    nc.compile()
    return nc


def _pow2floor(x):
    return float(2.0 ** np.floor(np.log2(x)))


def _q8(x, s):
    return np.clip(np.asarray(x, np.float64) * s, -240.0, 240.0).astype(
        ml_dtypes.float8_e4m3)


def _pack_rows(w8):
    """[(a*128+p), n] -> [p, (a n)] packed fp8 array."""
    a = w8.shape[0] // 128
    return np.ascontiguousarray(
        w8.reshape(a, 128, -1).transpose(1, 0, 2).reshape(128, -1))


def _scales(inputs):
    f64 = np.float64
    scale = DH ** (-0.5)
    tanh_a = np.tanh(f64(inputs["attn_gate"][0]))
    tanh_f = np.tanh(f64(inputs["ff_gate"][0]))
    Wg = inputs["ln_g"].astype(f64)[:, None] * inputs["Wq"].astype(f64) * scale
    W1g = inputs["ff_ln_g"].astype(f64)[:, None] * inputs["W1"].astype(f64)
    SQ = _pow2floor(224.0 / np.abs(Wg).max())
    SKV = _pow2floor(224.0 / np.abs(inputs["Wkv"]).max())
    S1 = _pow2floor(224.0 / np.abs(W1g).max())
    S2 = min(_pow2floor(224.0 / np.abs(inputs["Wout"] * tanh_a).max()),
             _pow2floor(224.0 / np.abs(inputs["W2"] * tanh_f).max()))
    return SQ, SKV, S1, S2, Wg, W1g, tanh_a, tanh_f


def _prep_in_maps(inputs, SQ, SKV, S1, S2, Wg, W1g, tanh_a, tanh_f):
    bf = ml_dtypes.bfloat16
    f64 = np.float64
    scale = DH ** (-0.5)
    qo = inputs["qo"]
    kvo = inputs["kvo"]
    attn_mask = inputs["attn_mask"]
    q_mask = inputs["q_mask"]
    kv_mask = inputs["kv_mask"]

    wgq = _q8(Wg, SQ)
    cwg = wgq.astype(np.float32).sum(axis=0)[None, :]
    wqv = (inputs["ln_b"].astype(f64) @ inputs["Wq"].astype(f64) * scale)
    wkvq = _q8(inputs["Wkv"], SKV)
    woq = _q8(inputs["Wout"].astype(f64) * tanh_a * S2, 1.0)
    w1q = _q8(W1g, S1)
    cw1 = w1q.astype(np.float32).sum(axis=0)[None, :]
    w1v = (inputs["ff_ln_b"].astype(f64) @ inputs["W1"].astype(f64))
    w2q = _q8(inputs["W2"].astype(f64) * tanh_f * S2, 1.0)
    # w2 packed [p, (e a n)]
    w2p = np.ascontiguousarray(
        w2q.reshape(32, 128, 8, 128).transpose(1, 2, 0, 3).reshape(128, -1))
    shared = {
        "wgq": _pack_rows(wgq),
        "cwg": np.ascontiguousarray(cwg, dtype=bf),
        "wqv": np.ascontiguousarray(wqv.reshape(4, 128).T, dtype=np.float32),
        "wkvq": _pack_rows(wkvq),
        "woq": _pack_rows(woq),
        "w1q": _pack_rows(w1q),
        "cw1": np.ascontiguousarray(cw1, dtype=bf),
        "w1v": np.ascontiguousarray(w1v.reshape(32, 128).T,
                                    dtype=np.float32),
        "w2q": w2p,
    }
    in_maps = []
    for c in range(8):
        b, hf = c // 2, c % 2
        rows = slice(hf * TI, (hf + 1) * TI)
        m = (attn_mask[b, rows, :] & kv_mask[b].reshape(J)[None, :])
        mask01 = _pack_rows(m.T.astype(ml_dtypes.float8_e4m3))
        kvoT = np.asarray(kvo[b], np.float32).reshape(J, DL).T
        im = dict(shared)
        im["qoT"] = np.ascontiguousarray(qo[b, rows, :].T * np.float32(S2),
                                         dtype=bf)
        im["kvq"] = _pack_rows(_q8(kvoT, 1.0))
        im["mask01"] = mask01
        im["qmaskT"] = np.ascontiguousarray(q_mask[b, rows][None, :],
                                            dtype=np.float32)
        in_maps.append(im)
    return in_maps


def kernel(**inputs):
    global _nc_cache, _nc_key
    inputs = {k: np.asarray(v) for k, v in inputs.items()}
    SQ, SKV, S1, S2, Wg, W1g, tanh_a, tanh_f = _scales(inputs)
    in_maps = _prep_in_maps(inputs, SQ, SKV, S1, S2, Wg, W1g, tanh_a, tanh_f)
    key = (SQ, SKV, S1, S2)
    if _nc_cache is None or _nc_key != key:
        _nc_cache = build_nc(SQ, SKV, S1, S2)
        _nc_key = key
    from concourse.bass_utils import run_bass_kernel_spmd
    res = run_bass_kernel_spmd(_nc_cache, in_maps, list(range(8)))
    out = np.empty((B, T1, DIM), dtype=np.float32)
    inv = np.float32(1.0 / S2)
    for c in range(8):
        b, hf = c // 2, c % 2
        out[b, hf * TI:(hf + 1) * TI, :] = res.results[c]["out"].T * inv
    return out


if __name__ == "__main__":
    nc = build_nc(2.0 ** 14, 2.0 ** 11, 2.0 ** 13, 2.0 ** 14)
    print("built ok")


# revision 30
# speedup vs baseline: 1.5896x; 1.1105x over previous
"""GatedCrossAttentionBlock Trainium2 kernel, SPMD over 8 NeuronCores.

Sharding: core c handles batch b=c//2, T1-half h=c%2 (1024 rows of T1).
No collectives. Activations feature-major (transposed); all big matmuls
fp8e4 DoubleRow (2x tensor throughput), accumulating f32 in PSUM.

Scale folding: the whole post-attention residual stream is carried
S2-scaled (S2 a power of two) so Wout/W2 quantization scales cost no
extra ops; host divides the output by S2. LayerNorm mean-subtraction is
folded into the projection matmuls as a rank-1 update (colsum(W) x
mu*rstd), so normalize is a single vector multiply per tile.
"""
import sys

for _p in ("/opt/trn_rl_repo", "/root/.axon_site/_ro/trn_rl_repo"):
    if _p not in sys.path:
        sys.path.insert(0, _p)

import numpy as np
import ml_dtypes
from contextlib import ExitStack

import concourse.bass as bass
from concourse import bacc
import concourse.mybir as mybir
import concourse.tile as tile

F32 = mybir.dt.float32
BF16 = mybir.dt.bfloat16
FP8 = mybir.dt.float8e4
AF = mybir.ActivationFunctionType
ALU = mybir.AluOpType
DR = mybir.MatmulPerfMode.DoubleRow

B, T1, TKV, N_, DIM, DL, DH, H, MULT = 4, 2048, 8, 64, 1024, 1024, 64, 8, 4
J = TKV * N_          # 512
INNER = H * DH        # 512
DFF = MULT * DIM      # 4096
TI = 1024             # T1 rows per core
NBLK = 2              # i-blocks of 512 per core
CT = DIM // 128       # 8 c-tiles
TINY = 1e-30
EPS = 1e-5

_nc_cache = None
_nc_key = None


def build_nc(SQ, SKV, S1, S2):
    nc = bacc.Bacc()
    d_qoT = nc.declare_dram_parameter("qoT", [DIM, TI], BF16, isOutput=False)
    d_kvq = nc.declare_dram_parameter("kvq", [128, 8 * J], FP8, isOutput=False)
    d_mask = nc.declare_dram_parameter("mask01", [128, 4 * TI], FP8,
                                       isOutput=False)
    d_qm = nc.declare_dram_parameter("qmaskT", [1, TI], F32, isOutput=False)
    d_wgq = nc.declare_dram_parameter("wgq", [128, 8 * INNER], FP8,
                                      isOutput=False)
    d_cwg = nc.declare_dram_parameter("cwg", [1, 2 * INNER], FP8,
                                      isOutput=False)
    d_wqv = nc.declare_dram_parameter("wqv", [128, 4], F32, isOutput=False)
    d_wkvq = nc.declare_dram_parameter("wkvq", [128, 8 * 2 * INNER], FP8,
                                       isOutput=False)
    d_woq = nc.declare_dram_parameter("woq", [128, 4 * DIM], FP8,
                                      isOutput=False)
    d_w1q = nc.declare_dram_parameter("w1q", [128, 8 * DFF], FP8,
                                      isOutput=False)
    d_cw1 = nc.declare_dram_parameter("cw1", [1, 2 * DFF], FP8,
                                      isOutput=False)
    d_w1v = nc.declare_dram_parameter("w1v", [128, 32], F32, isOutput=False)
    d_w2q = nc.declare_dram_parameter("w2q", [128, 8 * 32 * 128], FP8,
                                      isOutput=False)
    d_out = nc.declare_dram_parameter("out", [DIM, TI], F32, isOutput=True)

    with tile.TileContext(nc) as tc, ExitStack() as ctx:
        pers = ctx.enter_context(tc.tile_pool(name="pers", bufs=1))
        # ---------------- persistent tiles ----------------
        xT = [pers.tile([128, TI], F32, tag=f"xT{t}", name=f"xT{t}")
              for t in range(CT)]
        # LN output, fp8, DoubleRow layout: tile tp holds chunks 2tp, 2tp+1.
        xc8 = [pers.tile([128, 2, TI], FP8, tag=f"xc{t}", name=f"xc{t}")
               for t in range(4)]
        w1q_sb = pers.tile([128, 8, DFF], FP8, tag="w1q", name="w1q_sb")
        cwg_sb = pers.tile([1, 2, INNER], FP8, tag="cwg", name="cwg_sb")
        cw1_sb = pers.tile([1, 2, DFF], FP8, tag="cw1", name="cw1_sb")
        wqv_sb = pers.tile([128, 4], F32, tag="wqv", name="wqv_sb")
        w1v_sb = pers.tile([128, 32], F32, tag="w1v", name="w1v_sb")
        qm_sb = pers.tile([1, TI], F32, tag="qm", name="qm_sb")
        ones_c = pers.tile([128, 1], BF16, tag="ones_c", name="ones_c")
        ones_r = pers.tile([1, 128], BF16, tag="ones_r", name="ones_r")
        eps_t = pers.tile([1, 1], F32, tag="eps_t", name="eps_t")
        # rank-1 LN mean-correction operand: fp8 pairs [(−mu·rstd·64), 0]
        # so the correction matmul rides the DoubleRow path with cw/64.
        negmurb = [pers.tile([1, 2, TI], FP8, tag=f"nmr{i}", name=f"nmr{i}")
                   for i in range(2)]
        for i in range(2):
            nc.vector.memset(negmurb[i][:, 1, :], 0.0)
        nc.vector.memset(ones_c[:], 1.0)
        nc.vector.memset(ones_r[:], 1.0)
        nc.vector.memset(eps_t[:], EPS * S2 * S2)

        scr = ctx.enter_context(tc.tile_pool(name="scr", bufs=3))

        def ln_rank1(pa, ps_stat, src_tiles, src_bf, rb_sb, nmr, tag):
            """Stats of a feature-major (DIM, TI) S2-scaled activation.
            Writes rb_sb [128, TI] f32 (rstd broadcast) and nmr [1, TI]
            bf16 (-mu*rstd) for the rank-1 mean correction."""
            st = [ps_stat.tile([33, 512], F32, tag=f"stat{b}",
                               name=f"st{tag}{b}") for b in range(NBLK)]
            for t in range(CT):
                if src_bf:
                    cbf = src_tiles[t]
                else:
                    cbf = scr.tile([128, TI], BF16, tag="statbf",
                                   name="statbf", bufs=2)
                    nc.vector.tensor_copy(cbf[:], src_tiles[t][:])
                sq = scr.tile([128, TI], BF16, tag="statsq", name="statsq",
                              bufs=2)
                nc.scalar.square(sq[:], cbf[:])
                for b in range(NBLK):
                    sl = slice(b * 512, b * 512 + 512)
                    nc.tensor.matmul(st[b][0:1, :], ones_c[:], cbf[:, sl],
                                     start=(t == 0), stop=(t == CT - 1))
                    nc.tensor.matmul(st[b][32:33, :], ones_c[:],
                                     sq[:, sl], start=(t == 0),
                                     stop=(t == CT - 1))
            mu = pa.tile([1, TI], F32, tag="st_mu", name=f"mu{tag}")
            ex2 = pa.tile([1, TI], F32, tag="st_ex2", name=f"ex2{tag}")
            for b in range(NBLK):
                sl = slice(b * 512, b * 512 + 512)
                nc.vector.tensor_scalar_mul(mu[:, sl], st[b][0:1, :],
                                            1.0 / DIM)
                nc.vector.tensor_scalar_mul(ex2[:, sl], st[b][32:33, :],
                                            1.0 / DIM)
            musq = pa.tile([1, TI], F32, tag="st_musq", name=f"musq{tag}")
            nc.vector.tensor_mul(musq[:], mu[:], mu[:])
            var = pa.tile([1, TI], F32, tag="st_var", name=f"var{tag}")
            nc.vector.tensor_sub(var[:], ex2[:], musq[:])
            std = pa.tile([1, TI], F32, tag="st_musq", name=f"std{tag}")
            nc.scalar.activation(std[:], var[:], AF.Sqrt, bias=eps_t[:])
            r = pa.tile([1, TI], F32, tag="st_ex2", name=f"r{tag}")
            nc.vector.reciprocal_approx_fast(r[:], std[:])
            r_bf = pa.tile([1, TI], BF16, tag="st_rbf", name=f"rbf{tag}")
            nc.vector.tensor_copy(r_bf[:], r[:])
            # nmr pair 0 = -mu * rstd * 64 (cw is pre-divided by 64)
            nmrf = pa.tile([1, TI], F32, tag="st_var", name=f"nmrf{tag}")
            nc.vector.tensor_mul(nmrf[:], mu[:], r[:])
            nc.vector.tensor_scalar_mul(nmr[:, 0, :], nmrf[:], -64.0)
            for b in range(NBLK):
                sl = slice(b * 512, b * 512 + 512)
                rb_ps = ps_stat.tile([128, 512], F32, tag="rbb",
                                     name=f"rbps{tag}{b}", bufs=2)
                nc.tensor.matmul(rb_ps[:], ones_r[:], r_bf[:, sl],
                                 start=True, stop=True)
                nc.vector.tensor_copy(rb_sb[:, sl], rb_ps[:])

        def norm_mul(src_tiles, rb_sb):
            for t in range(CT):
                nc.vector.tensor_mul(xc8[t // 2][:, t % 2, :],
                                     src_tiles[t][:], rb_sb[:])

        with tc.tile_pool(name="attn", bufs=1) as pa:
            qoT = [pa.tile([128, TI], BF16, tag=f"qoT{t}", name=f"qoT{t}")
                   for t in range(CT)]
            kv_sb = pa.tile([128, 8, J], FP8, tag="kv", name="kv_sb")
            mask_sb = pa.tile([128, 4, TI], FP8, tag="mask", name="mask_sb")
            wgq_sb = pa.tile([128, 8, INNER], FP8, tag="wgq", name="wgq_sb")
            wkvq_sb = pa.tile([128, 8, 2 * INNER], FP8, tag="wkvq",
                              name="wkvq_sb")
            woq_sb = pa.tile([128, 4, DIM], FP8, tag="woq", name="woq_sb")
            rb1_sb = pa.tile([128, TI], F32, tag="rb1", name="rb1_sb")
            rb2_sb = pa.tile([128, TI], F32, tag="rb1", name="rb2_sb")
            # plane layout: tile g, partition 32m+p, pair-index i holds
            # head 4g+m, dh=32i+p — so sim contracts dh as 32 partitions x 2
            # DoubleRow subtiles (weights are column-permuted host-side).
            qT8 = [pa.tile([128, 2, TI], FP8, tag=f"qT{g}", name=f"qT{g}")
                   for g in range(2)]
            kT8 = [pa.tile([128, 2, J], FP8, tag=f"kT{g}", name=f"kT{g}")
                   for g in range(2)]
            # per-head stride padded to 72 so DoubleRow ldweights APs stay
            # even-sized and even-aligned (65 is rejected by codegen)
            VP = 72
            v_aug = [pa.tile([128, 2, H, VP], FP8, tag=f"vaug{j}",
                             name=f"vaug{j}") for j in range(2)]
            attn_cat = [pa.tile([128, 2, TI], FP8, tag=f"acat{d}",
                                name=f"acat{d}") for d in range(2)]

            for t in range(CT):
                nc.sync.dma_start(out=qoT[t],
                                  in_=d_qoT[t * 128:(t + 1) * 128, :])
            nc.sync.dma_start(out=kv_sb,
                              in_=d_kvq.rearrange("p (a j) -> p a j", a=8))
            nc.sync.dma_start(out=mask_sb,
                              in_=d_mask.rearrange("p (a t) -> p a t", a=4))
            nc.sync.dma_start(out=qm_sb, in_=d_qm[:, :])
            nc.sync.dma_start(out=wgq_sb,
                              in_=d_wgq.rearrange("p (a n) -> p a n", a=8))
            nc.sync.dma_start(out=wkvq_sb,
                              in_=d_wkvq.rearrange("p (a n) -> p a n", a=8))
            nc.sync.dma_start(out=cwg_sb,
                              in_=d_cwg.rearrange("p (a n) -> p a n", a=2))
            nc.sync.dma_start(out=wqv_sb, in_=d_wqv[:, :])
            nc.sync.dma_start(out=cw1_sb,
                              in_=d_cw1.rearrange("p (a n) -> p a n", a=2))
            nc.sync.dma_start(out=w1v_sb, in_=d_w1v[:, :])
            nc.sync.dma_start(out=woq_sb,
                              in_=d_woq.rearrange("p (a n) -> p a n", a=4))
            nc.sync.dma_start(out=w1q_sb,
                              in_=d_w1q.rearrange("p (a n) -> p a n", a=8))

            for jp in range(2):
                nc.vector.memset(v_aug[jp][:, :, :, DH:DH + 1], 1.0)
                nc.vector.memset(v_aug[jp][:, :, :, DH + 1:VP], 0.0)

            # ---- LN1 stats + k/v projections ----
            with tc.tile_pool(name="psStat", bufs=1, space="PSUM") as psStat, \
                 tc.tile_pool(name="psKV", bufs=2, space="PSUM") as psKV:
                ln_rank1(pa, psStat, qoT, True, rb1_sb, negmurb[0], "1")
                for d in range(4):
                    k_ps = psKV.tile([128, 512], F32, tag="kv", name="k_ps")
                    for tp in range(4):
                        nc.tensor.matmul(
                            k_ps[:],
                            wkvq_sb[:, 2 * tp:2 * tp + 2,
                                    d * 128:(d + 1) * 128],
                            kv_sb[:, 2 * tp:2 * tp + 2, :],
                            start=(tp == 0), stop=(tp == 3), perf_mode=DR)
                    nc.scalar.activation(kT8[d // 2][:, d % 2, :], k_ps[:],
                                         AF.Copy, scale=1.0 / SKV)
                for c in range(4):
                    v_ps = psKV.tile([128, 512], F32, tag="kv", name="v_ps")
                    for tp in range(4):
                        nc.tensor.matmul(
                            v_ps[:],
                            kv_sb[:, 2 * tp:2 * tp + 2,
                                  c * 128:(c + 1) * 128],
                            wkvq_sb[:, 2 * tp:2 * tp + 2, INNER:2 * INNER],
                            start=(tp == 0), stop=(tp == 3), perf_mode=DR)
                    nc.vector.tensor_scalar_mul(
                        v_aug[c // 2][:, c % 2, :, 0:DH],
                        v_ps[:].rearrange("p (h d) -> p h d", h=H),
                        1.0 / SKV)
            norm_mul(qoT, rb1_sb)

            # ---- q projection ----
            with tc.tile_pool(name="psQ", bufs=2, space="PSUM") as psQ:
                for d in range(4):
                    q_ps = psQ.tile([128, 2, 512], F32, tag="q", name="q_ps")
                    for b in range(NBLK):
                        sl = slice(b * 512, b * 512 + 512)
                        for tp in range(4):
                            nc.tensor.matmul(
                                q_ps[:, b, :],
                                wgq_sb[:, 2 * tp:2 * tp + 2,
                                       d * 128:(d + 1) * 128],
                                xc8[tp][:, :, sl],
                                start=(tp == 0), stop=False, perf_mode=DR)
                        nc.tensor.matmul(q_ps[:, b, :],
                                         cwg_sb[:, :, d * 128:(d + 1) * 128],
                                         negmurb[0][:, :, sl],
                                         start=False, stop=True, perf_mode=DR)
                    for b in range(NBLK):
                        sl = slice(b * 512, b * 512 + 512)
                        nc.vector.tensor_scalar(qT8[d // 2][:, d % 2, sl],
                                                q_ps[:, b, :],
                                                1.0 / SQ, wqv_sb[:, d:d + 1],
                                                op0=ALU.mult, op1=ALU.add)

            # ---- attention ----
            with tc.tile_pool(name="psS", bufs=2, space="PSUM") as psS, \
                 tc.tile_pool(name="psAv", bufs=2, space="PSUM") as psAv:
                for h in range(H):
                    g, m = h // 4, h % 4
                    pr = slice(32 * m, 32 * m + 32)
                    row = 64 * (h % 2)
                    for b in range(NBLK):
                        sl = slice(b * 512, b * 512 + 512)
                        av_ps = psAv.tile([VP, 512], F32, tag="av",
                                          name="av_ps")
                        for jp in range(2):
                            s_ps = psS.tile([128, 2, 512], F32, tag="sim",
                                            name="s_ps")
                            for i in range(2):
                                jc = 2 * jp + i
                                nc.tensor.matmul(
                                    s_ps[:, i, :],
                                    kT8[g][pr, :, jc * 128:(jc + 1) * 128],
                                    qT8[g][pr, :, sl],
                                    start=True, stop=True, perf_mode=DR,
                                    tile_position=(32 * m, 0))
                            pe = scr.tile([128, 2, 512], BF16, tag="pe",
                                          name="pe", bufs=2)
                            nc.scalar.activation(pe[:], s_ps[:], AF.Exp)
                            pq = scr.tile([128, 2, 512], FP8, tag="pq",
                                          name="pq", bufs=3)
                            nc.gpsimd.tensor_mul(
                                pq[:], pe[:], mask_sb[:, 2 * jp:2 * jp + 2, sl])
                            nc.tensor.matmul(av_ps[:],
                                             v_aug[jp][:, :, h, :], pq[:],
                                             start=(jp == 0), stop=(jp == 1),
                                             perf_mode=DR)
                        s_t = scr.tile([1, 512], F32, tag="s_t", name="s_t",
                                       bufs=2)
                        nc.vector.tensor_scalar_add(s_t[:],
                                                    av_ps[DH:DH + 1, :], TINY)
                        rec = scr.tile([1, 512], F32, tag="rec", name="rec",
                                       bufs=2)
                        nc.vector.reciprocal_approx_fast(rec[:], s_t[:])
                        r_bf = scr.tile([1, 512], BF16, tag="rbf_h",
                                        name="rbf_h", bufs=2)
                        nc.vector.tensor_mul(r_bf[:], rec[:], qm_sb[:, sl])
                        rb_ps = psAv.tile([64, 512], F32, tag="rb",
                                          name="rb_ps")
                        nc.tensor.matmul(rb_ps[:], ones_r[:, 0:64], r_bf[:],
                                         start=True, stop=True)
                        rb2 = scr.tile([64, 512], F32, tag="rb2", name="rb2",
                                       bufs=2)
                        nc.vector.tensor_copy(rb2[:], rb_ps[:])
                        nc.vector.tensor_mul(
                            attn_cat[h // 4][row:row + 64, (h // 2) % 2, sl],
                            av_ps[0:DH, :], rb2[:])

            # ---- Wout + residual, then LN2 ----
            with tc.tile_pool(name="psC", bufs=2, space="PSUM") as psC, \
                 tc.tile_pool(name="psStat2", bufs=1, space="PSUM") as psS2:
                for b in range(NBLK):
                    sl = slice(b * 512, b * 512 + 512)
                    for e in range(CT):
                        wo_ps = psC.tile([128, 512], F32, tag="wo",
                                         name="wo_ps")
                        for dp in range(2):
                            nc.tensor.matmul(
                                wo_ps[:],
                                woq_sb[:, 2 * dp:2 * dp + 2,
                                       e * 128:(e + 1) * 128],
                                attn_cat[dp][:, :, sl],
                                start=(dp == 0), stop=(dp == 1), perf_mode=DR)
                        nc.vector.tensor_add(xT[e][:, sl], wo_ps[:],
                                             qoT[e][:, sl])
                ln_rank1(pa, psS2, xT, False, rb2_sb, negmurb[1], "2")
            norm_mul(xT, rb2_sb)

        # ---------------- FFN ----------------
        with tc.tile_pool(name="ffn", bufs=1) as pf, \
             tc.tile_pool(name="wstream", bufs=2) as ws, \
             tc.tile_pool(name="ostage", bufs=2) as ost, \
             tc.tile_pool(name="psH", bufs=3, space="PSUM") as psH:
            gT8 = [pf.tile([128, 2, TI], FP8, tag=f"gT{f}", name=f"gT{f}")
                   for f in range(16)]
            for f in range(32):
                h_ps = psH.tile([128, 2, 512], F32, tag="h", name="h_ps")
                for b in range(NBLK):
                    sl = slice(b * 512, b * 512 + 512)
                    for tp in range(4):
                        nc.tensor.matmul(
                            h_ps[:, b, :],
                            w1q_sb[:, 2 * tp:2 * tp + 2,
                                   f * 128:(f + 1) * 128],
                            xc8[tp][:, :, sl],
                            start=(tp == 0), stop=False, perf_mode=DR)
                    nc.tensor.matmul(h_ps[:, b, :],
                                     cw1_sb[:, :, f * 128:(f + 1) * 128],
                                     negmurb[1][:, :, sl],
                                     start=False, stop=True, perf_mode=DR)
                nc.scalar.activation(
                    gT8[f // 2][:, f % 2, :],
                    h_ps[:].rearrange("p a t -> p (a t)"),
                    AF.Gelu, bias=w1v_sb[:, f:f + 1], scale=1.0 / S1)
            w2r = d_w2q.rearrange("p (e a n) -> p e a n", e=8, a=32)
            for e in range(CT):
                w2t = ws.tile([128, 32, 128], FP8, tag="w2s", name="w2t")
                nc.sync.dma_start(out=w2t, in_=w2r[:, e])
                h2_ps = psH.tile([128, 2, 512], F32, tag="h", name="h2_ps")
                for b in range(NBLK):
                    sl = slice(b * 512, b * 512 + 512)
                    for fp in range(16):
                        nc.tensor.matmul(
                            h2_ps[:, b, :],
                            w2t[:, 2 * fp:2 * fp + 2, :],
                            gT8[fp][:, :, sl],
                            start=(fp == 0), stop=(fp == 15), perf_mode=DR)
                stg = ost.tile([128, TI], F32, tag="stg", name="stg")
                nc.vector.tensor_add(stg[:],
                                     h2_ps[:].rearrange("p a t -> p (a t)"),
                                     xT[e][:])
                nc.sync.dma_start(out=d_out[e * 128:(e + 1) * 128, :],
                                  in_=stg[:])
    nc.compile()
    return nc


def _pow2floor(x):
    return float(2.0 ** np.floor(np.log2(x)))


def _q8(x, s):
    return np.clip(np.asarray(x, np.float64) * s, -240.0, 240.0).astype(
        ml_dtypes.float8_e4m3)


def _pack_rows(w8):
    """[(a*128+p), n] -> [p, (a n)] packed fp8 array."""
    a = w8.shape[0] // 128
    return np.ascontiguousarray(
        w8.reshape(a, 128, -1).transpose(1, 0, 2).reshape(128, -1))


def _scales(inputs):
    f64 = np.float64
    scale = DH ** (-0.5)
    tanh_a = np.tanh(f64(inputs["attn_gate"][0]))
    tanh_f = np.tanh(f64(inputs["ff_gate"][0]))
    Wg = inputs["ln_g"].astype(f64)[:, None] * inputs["Wq"].astype(f64) * scale
    W1g = inputs["ff_ln_g"].astype(f64)[:, None] * inputs["W1"].astype(f64)
    SQ = _pow2floor(224.0 / np.abs(Wg).max())
    SKV = _pow2floor(224.0 / np.abs(inputs["Wkv"]).max())
    S1 = _pow2floor(224.0 / np.abs(W1g).max())
    S2 = min(_pow2floor(224.0 / np.abs(inputs["Wout"] * tanh_a).max()),
             _pow2floor(224.0 / np.abs(inputs["W2"] * tanh_f).max()))
    return SQ, SKV, S1, S2, Wg, W1g, tanh_a, tanh_f


def _prep_in_maps(inputs, SQ, SKV, S1, S2, Wg, W1g, tanh_a, tanh_f):
    bf = ml_dtypes.bfloat16
    f64 = np.float64
    scale = DH ** (-0.5)
    qo = inputs["qo"]
    kvo = inputs["kvo"]
    attn_mask = inputs["attn_mask"]
    q_mask = inputs["q_mask"]
    kv_mask = inputs["kv_mask"]

    # plane permutation: old col n = h*64+dh -> new col (2g+i)*128+32m+p
    # with h=4g+m, dh=32i+p (sim contracts dh via 32 partitions x 2 DR)
    n = np.arange(INNER)
    h_, dh_ = n // 64, n % 64
    newidx = (2 * (h_ // 4) + dh_ // 32) * 128 + 32 * (h_ % 4) + dh_ % 32
    Wg_p = np.empty_like(Wg)
    Wg_p[:, newidx] = Wg
    wgq = _q8(Wg_p, SQ)
    cwg = wgq.astype(np.float32).sum(axis=0)
    cw8g = np.zeros((1, 2 * INNER), dtype=ml_dtypes.float8_e4m3)
    cw8g[0, :INNER] = _q8(cwg / 64.0, 1.0)
    wqv = (inputs["ln_b"].astype(f64) @ inputs["Wq"].astype(f64) * scale)
    wqv_p = np.empty_like(wqv)
    wqv_p[newidx] = wqv
    Wkv_p = np.array(inputs["Wkv"], dtype=f64)
    Wkv_p[:, newidx] = Wkv_p[:, :INNER].copy()
    wkvq = _q8(Wkv_p, SKV)
    woq = _q8(inputs["Wout"].astype(f64) * tanh_a * S2, 1.0)
    w1q = _q8(W1g, S1)
    cw1 = w1q.astype(np.float32).sum(axis=0)
    cw81 = np.zeros((1, 2 * DFF), dtype=ml_dtypes.float8_e4m3)
    cw81[0, :DFF] = _q8(cw1 / 64.0, 1.0)
    w1v = (inputs["ff_ln_b"].astype(f64) @ inputs["W1"].astype(f64))
    w2q = _q8(inputs["W2"].astype(f64) * tanh_f * S2, 1.0)
    # w2 packed [p, (e a n)]
    w2p = np.ascontiguousarray(
        w2q.reshape(32, 128, 8, 128).transpose(1, 2, 0, 3).reshape(128, -1))
    shared = {
        "wgq": _pack_rows(wgq),
        "cwg": cw8g,
        "wqv": np.ascontiguousarray(wqv_p.reshape(4, 128).T,
                                    dtype=np.float32),
        "wkvq": _pack_rows(wkvq),
        "woq": _pack_rows(woq),
        "w1q": _pack_rows(w1q),
        "cw1": cw81,
        "w1v": np.ascontiguousarray(w1v.reshape(32, 128).T,
                                    dtype=np.float32),
        "w2q": w2p,
    }
    in_maps = []
    for c in range(8):
        b, hf = c // 2, c % 2
        rows = slice(hf * TI, (hf + 1) * TI)
        m = (attn_mask[b, rows, :] & kv_mask[b].reshape(J)[None, :])
        mask01 = _pack_rows(m.T.astype(ml_dtypes.float8_e4m3))
        kvoT = np.asarray(kvo[b], np.float32).reshape(J, DL).T
        im = dict(shared)
        im["qoT"] = np.ascontiguousarray(qo[b, rows, :].T * np.float32(S2),
                                         dtype=bf)
        im["kvq"] = _pack_rows(_q8(kvoT, 1.0))
        im["mask01"] = mask01
        im["qmaskT"] = np.ascontiguousarray(q_mask[b, rows][None, :],
                                            dtype=np.float32)
        in_maps.append(im)
    return in_maps


def kernel(**inputs):
    global _nc_cache, _nc_key
    inputs = {k: np.asarray(v) for k, v in inputs.items()}
    SQ, SKV, S1, S2, Wg, W1g, tanh_a, tanh_f = _scales(inputs)
    in_maps = _prep_in_maps(inputs, SQ, SKV, S1, S2, Wg, W1g, tanh_a, tanh_f)
    key = (SQ, SKV, S1, S2)
    if _nc_cache is None or _nc_key != key:
        _nc_cache = build_nc(SQ, SKV, S1, S2)
        _nc_key = key
    from concourse.bass_utils import run_bass_kernel_spmd
    res = run_bass_kernel_spmd(_nc_cache, in_maps, list(range(8)))
    out = np.empty((B, T1, DIM), dtype=np.float32)
    inv = np.float32(1.0 / S2)
    for c in range(8):
        b, hf = c // 2, c % 2
        out[b, hf * TI:(hf + 1) * TI, :] = res.results[c]["out"].T * inv
    return out


if __name__ == "__main__":
    nc = build_nc(2.0 ** 14, 2.0 ** 11, 2.0 ** 13, 2.0 ** 14)
    print("built ok")
